# revision 52
# baseline (speedup 1.0000x reference)
"""Trainium2 Bass kernel for the EnhancedGATBlock problem.

Strategy (node/window sharded, no collectives):
  - Host sorts edges by dst and greedily packs consecutive dst-nodes into
    "windows" of <=128 nodes and <=KSUB*128 edges.  Every incoming edge of a
    node lives in exactly one window, so each window's segment-softmax and
    aggregation are fully local.
  - Windows are dealt round-robin onto 8 NeuronCores; every core runs an
    IDENTICAL static schedule of W windows x KSUB subtiles of 128 edges
    (required because run_bass_kernel_spmd compiles one SPMD program).  All
    data-dependence (edge->window assignment, node ids, per-window slots) is
    carried in index arrays, never in the IR.
  - Softmax uses a fixed shift C instead of the per-node max; alpha is
    mathematically invariant to the shift and exp stays comfortably inside
    f32 range for this data distribution (logits ~ [-12, 12]).
  - Per subtile on device: indirect-DMA gather of xl[src] rows, one-hot
    (edge x node-slot) matrix built by iota-compare, then one PSUM
    accumulation group of three matmuls (ee from host-transposed edge_attr,
    one-hot xr-expand, identity-add of xl), leaky-relu + att-dot + exp, and
    a single scatter matmul accumulating [nodes, msg|denom] into PSUM.
"""
import numpy as np

import concourse.bass as bass
import concourse.tile as tile
import concourse.mybir as mybir
from concourse import library_config
from concourse.bass_utils import run_bass_kernel_spmd

# ---- problem constants (hardcoded per the grading contract) ----
N, E = 50000, 800000
IN_DIM, HID, HEADS, EDGE_DIM = 64, 64, 4, 32
F = HEADS * HID            # 256
NEG_SLOPE = 0.2
LN_EPS = 1e-5

P = 128
NCORES = 8
KSUB = 16                  # subtiles (of 128 edges) per window
EPW = P * KSUB             # edges per window
C_SHIFT = 12.0             # fixed softmax shift (see module docstring)
DENOM_TINY = 1e-30         # guards 0-degree / pad node slots against 0/0
NMETA = 2 * KSUB + 1       # src idx | dst slot | window node id

FP = mybir.dt.float32
FR = mybir.dt.float32r     # fast PE path (1 cycle/row at N>=256)
BF = mybir.dt.bfloat16
I32 = mybir.dt.int32
ALU = mybir.AluOpType
ACT = mybir.ActivationFunctionType
AX = mybir.AxisListType


# --------------------------------------------------------------------------
# host-side prep
# --------------------------------------------------------------------------

def _pack_windows_ab(degA, degB, half_cap):
    """Greedy pack consecutive nodes into windows such that each window has
    <=128 nodes, <=half_cap edges with src in table-half A and likewise for
    half B (each half is gathered by one int16 dma_gather call)."""
    wins = []
    cur_nodes = 0
    ca = 0
    cb = 0
    start = 0
    for n in range(len(degA)):
        da, db = int(degA[n]), int(degB[n])
        assert da <= half_cap and db <= half_cap
        if cur_nodes + 1 > P or ca + da > half_cap or cb + db > half_cap:
            wins.append((start, n))
            start = n
            cur_nodes, ca, cb = 0, 0, 0
        cur_nodes += 1
        ca += da
        cb += db
    wins.append((start, len(degA)))
    return wins


def host_prep(edge_index, edge_attr, n_nodes=N):
    half_cap = EPW // 2                     # 1024 edges per table half
    HSPLIT = n_nodes // 2                   # xl table split row (int16 range)
    src = np.asarray(edge_index[0]).astype(np.int64)
    dst = np.asarray(edge_index[1]).astype(np.int64)
    # sort edges by (dst, src-half) so each window is [A-edges | B-edges]
    is_b = (src >= HSPLIT).astype(np.int64)
    order = np.lexsort((is_b, dst))
    dst_s = dst[order]
    is_b_s = is_b[order]
    deg = np.bincount(dst_s, minlength=n_nodes)
    degB = np.bincount(dst_s[is_b_s == 1], minlength=n_nodes)
    degA = deg - degB
    node_edge_start = np.concatenate([[0], np.cumsum(deg)])
    wins = _pack_windows_ab(degA, degB, half_cap)
    WT = len(wins)
    W = (WT + NCORES - 1) // NCORES

    GW = half_cap // 16                     # idx cols per half (wrapped by 16)
    meta = np.zeros((NCORES, W, P, NMETA), np.int32)
    meta[:, :, :, KSUB:2 * KSUB] = -1       # dst slot pad -> no OH match
    gidx = np.zeros((NCORES, W, 16, 2 * GW), np.int16)
    eat = np.zeros((NCORES, W, EDGE_DIM, EPW), np.float32)
    win_nodes_m = np.full((NCORES, W, P), -1, np.int64)  # pad -> -1

    edge_attr = np.asarray(edge_attr, np.float32)

    for widx, (a, b) in enumerate(wins):
        c = widx % NCORES
        w = widx // NCORES
        es, ee_ = int(node_edge_start[a]), int(node_edge_start[b])
        pe = order[es:ee_]
        bmask = is_b_s[es:ee_] == 1
        peA, peB = pe[~bmask], pe[bmask]
        nA, nB = len(peA), len(peB)
        # window-local edge positions: A block then B block at half_cap
        k = np.concatenate([np.arange(nA), half_cap + np.arange(nB)])
        pe2 = np.concatenate([peA, peB])
        p_pos = k % P
        j_pos = k // P
        meta[c, w, p_pos, j_pos] = src[pe2].astype(np.int32)
        meta[c, w, p_pos, KSUB + j_pos] = (dst[pe2] - a).astype(np.int32)
        # wrapped int16 gather indices (idx for slot k lives at [k%16, k//16])
        ia = (src[peA]).astype(np.int16)
        ib = (src[peB] - HSPLIT).astype(np.int16)
        gidx[c, w, np.arange(nA) % 16, np.arange(nA) // 16] = ia
        gidx[c, w, np.arange(nB) % 16, GW + np.arange(nB) // 16] = ib
        # c,w,k advanced indices with a slice between -> result dims are
        # (cnt, EDGE_DIM), matching edge_attr[pe2] directly
        eat[c, w, :, k] = edge_attr[pe2]
        nn = b - a
        meta[c, w, :nn, 2 * KSUB] = np.arange(a, b, dtype=np.int32)
        win_nodes_m[c, w, :nn] = np.arange(a, b)

    # dma_gather reads its wrapped index block from 128 partitions (the
    # 16-partition pattern replicated for the 8 gpsimd cores)
    gidx = np.tile(gidx, (1, 1, 8, 1))
    return dict(meta=meta, gidx=gidx, eat=eat, win_nodes_m=win_nodes_m,
                W=W, WT=WT, hsplit=HSPLIT)


# --------------------------------------------------------------------------
# BIR sync-wait legalization
# --------------------------------------------------------------------------
# walrus codegen accepts only ONE semaphore wait per ISA instruction, but
# Tile freely attaches more.  Keep the first wait on the instruction and move
# the excess onto preceding same-engine Drains (engines execute their stream
# in order, so the semantics are unchanged).

_SPILL_OPCODE = "Drain"


def legalize_sync_waits(bir_bytes):
    import orjson
    bir = orjson.loads(bir_bytes)
    n_new = 0
    for fn in bir["functions"]:
        for blk in fn["blocks"]:
            insts = blk.get("instructions")
            if not insts:
                continue
            out = []
            changed = False
            for ins in insts:
                si = ins.get("sync_info")
                waits = (si or {}).get("on_wait") or []
                if len(waits) > 1:
                    for wt in waits[1:]:
                        spill = {
                            "name": f"I-lsw{n_new}",
                            "opcode": _SPILL_OPCODE,
                            "engine": ins["engine"],
                            "ins": [],
                            "outs": [],
                            "sync_info": {"on_update": [], "on_wait": [wt]},
                        }
                        if "debug" in ins:
                            spill["debug"] = ins["debug"]
                        n_new += 1
                        out.append(spill)
                    si["on_wait"] = waits[:1]
                    changed = True
                out.append(ins)
            if changed:
                blk["instructions"] = out
    return orjson.dumps(bir)


def _patch_serialization(nc):
    orig = nc.to_json_bytes

    def patched():
        return legalize_sync_waits(orig())

    nc.to_json_bytes = patched
    return nc


# --------------------------------------------------------------------------
# device kernel
# --------------------------------------------------------------------------

def build_nc(W, n_nodes=N, use_prelu=True):
    nc = bass.Bass()
    xw_d = nc.declare_dram_parameter("xw", [n_nodes, IN_DIM], FP, isOutput=False)
    xt_d = nc.declare_dram_parameter("xt", [IN_DIM, n_nodes], FR, isOutput=False)
    wl_d = nc.declare_dram_parameter("wl", [IN_DIM, F], FP, isOutput=False)
    wr_d = nc.declare_dram_parameter("wr", [IN_DIM, F], FP, isOutput=False)
    we_d = nc.declare_dram_parameter("we", [EDGE_DIM, F], FP, isOutput=False)
    att_d = nc.declare_dram_parameter("att2", [1, F], FP, isOutput=False)
    cb_d = nc.declare_dram_parameter("cbias", [1, IN_DIM], FP, isOutput=False)
    cw_d = nc.declare_dram_parameter("clnw", [1, IN_DIM], FP, isOutput=False)
    clb_d = nc.declare_dram_parameter("clnb", [1, IN_DIM], FP, isOutput=False)
    ior_d = nc.declare_dram_parameter("iotar", [1, 4 * P], FP, isOutput=False)
    ioc_d = nc.declare_dram_parameter("iotac", [P, 1], FP, isOutput=False)
    meta_d = nc.declare_dram_parameter("meta", [W, P, NMETA], I32, isOutput=False)
    GW = (EPW // 2) // 16
    gid_d = nc.declare_dram_parameter("gidx", [W, P, 2 * GW], mybir.dt.int16,
                                      isOutput=False)
    eat_d = nc.declare_dram_parameter("eat", [W, EDGE_DIM, EPW], FR,
                                      isOutput=False)
    out_d = nc.declare_dram_parameter("out", [W * P, IN_DIM], FP, isOutput=True)
    xl_t_d = nc.dram_tensor("xl_table", [n_nodes, F], FR)

    with tile.TileContext(nc) as tc:
        with (
            tc.tile_pool(name="const", bufs=1) as cp,
            tc.tile_pool(name="win", bufs=2) as wp,
            tc.tile_pool(name="xlp", bufs=3) as xlp,
            tc.tile_pool(name="sub", bufs=3) as sp,
            tc.tile_pool(name="ep", bufs=2) as epp,
            tc.tile_pool(name="ptr", bufs=3, space="PSUM") as ptr,
            tc.tile_pool(name="pmm", bufs=3, space="PSUM") as pmm,
            tc.tile_pool(name="pout", bufs=2, space="PSUM") as pout,
        ):
            # ---------------- constants ----------------
            # Launder everything a matmul consumes through DVE so PE deps
            # collapse onto one semaphore (one-wait rule, see legalizer).
            def laundered(dram_ap, pdim, ncols, name, dt=FP):
                raw = cp.tile([pdim, ncols], FP, tag=name + "_r")
                nc.sync.dma_start(raw[:pdim, :], dram_ap)
                cl = cp.tile([pdim, ncols], dt, tag=name)
                nc.vector.tensor_copy(cl[:pdim, :], raw[:pdim, :])
                return cl

            wl_sb = laundered(wl_d[:, :], IN_DIM, F, "wl", dt=FR)
            wr_sb = laundered(wr_d[:, :], IN_DIM, F, "wr", dt=FR)
            we_sb = laundered(we_d[:, :], EDGE_DIM, F, "we", dt=FR)
            ior_sb = laundered(ior_d[:, :], 1, 4 * P, "ior")
            ioc_sb = laundered(ioc_d[:, :], P, 1, "ioc")
            ones1 = cp.tile([1, P], FP)
            nc.vector.memset(ones1[:], 1.0)

            def pbcast(src1, ncols, name):
                pb = pmm.tile([P, 4 * P], FP, tag="mm")
                nc.tensor.matmul(pb[:, :ncols], lhsT=ones1[:1, :],
                                 rhs=src1[:1, :ncols], start=True, stop=True)
                dst = cp.tile([P, ncols], FP, tag=name)
                nc.vector.tensor_copy(dst[:], pb[:, :ncols])
                return dst

            iota4 = pbcast(ior_sb, 4 * P, "iota4")   # [128, 512] four iotas
            ident = cp.tile([P, P], FP)
            nc.vector.tensor_tensor(out=ident[:],
                                    in0=ioc_sb[:, :1].to_broadcast([P, P]),
                                    in1=iota4[:, :P], op=ALU.is_equal)
            ident_r = cp.tile([P, P], FR)
            nc.vector.tensor_copy(ident_r[:], ident[:])
            att_rep = pbcast(laundered(att_d[:, :], 1, F, "att1"), F, "att_rep")
            att_bf = cp.tile([P, F], BF)
            nc.vector.tensor_copy(att_bf[:], att_rep[:])
            bias_rep = pbcast(laundered(cb_d[:, :], 1, IN_DIM, "b1"), IN_DIM,
                              "bias_rep")
            lnw_rep = pbcast(laundered(cw_d[:, :], 1, IN_DIM, "w1"), IN_DIM,
                             "lnw_rep")
            lnb_rep = pbcast(laundered(clb_d[:, :], 1, IN_DIM, "lb1"), IN_DIM,
                             "lnb_rep")
            czero = cp.tile([P, 1], FP)
            nc.vector.memset(czero[:], 0.0)
            cshift = cp.tile([P, 1], FP)
            nc.vector.memset(cshift[:], -C_SHIFT)
            ceps = cp.tile([P, 1], FP)
            nc.vector.memset(ceps[:], LN_EPS)

            # ---------------- prologue: xl table = x @ W_l ----------------
            # x arrives pre-transposed from the host, so each 128-node slice
            # is a direct lhsT; 512 nodes per DMA in and out.
            BN = 4 * P
            for b0 in range(0, n_nodes, BN):
                bcnt = min(BN, n_nodes - b0)
                nk = (bcnt + P - 1) // P
                xt_sb = xlp.tile([IN_DIM, BN], FR, tag="xts")
                nc.sync.dma_start(xt_sb[:IN_DIM, :bcnt],
                                  xt_d[:, b0:b0 + bcnt])
                xlo = xlp.tile([P, 4 * F], FR, tag="xlo")
                for k in range(nk):
                    cnt = min(P, bcnt - k * P)
                    pz = pmm.tile([P, F + HEADS], FP, tag="mm")
                    nc.tensor.matmul(
                        pz[:cnt, :F],
                        lhsT=xt_sb[:IN_DIM, k * P:k * P + cnt],
                        rhs=wl_sb[:, :], start=True, stop=True)
                    nc.vector.tensor_copy(xlo[:cnt, k * F:(k + 1) * F],
                                          pz[:cnt, :F])
                if bcnt == BN:
                    nc.sync.dma_start(
                        xl_t_d[b0:b0 + BN, :].rearrange("(k p) f -> p k f",
                                                        p=P),
                        xlo[:].rearrange("p (k f) -> p k f", k=4))
                else:
                    for k in range(nk):
                        cnt = min(P, bcnt - k * P)
                        nc.sync.dma_start(
                            xl_t_d[b0 + k * P:b0 + k * P + cnt, :],
                            xlo[:cnt, k * F:(k + 1) * F])

            # xl_table stores land on many DMA lanes; join once so the first
            # gather doesn't fan-in every lane.
            tc.strict_bb_all_engine_barrier()

            # ---------------- main loop over windows ----------------
            for w in range(W):
                meta_t = wp.tile([P, NMETA], I32, tag="meta")
                nc.sync.dma_start(meta_t[:], meta_d[w, :, :])
                dst_f = wp.tile([P, KSUB], FP, tag="dstf")
                nc.vector.tensor_copy(dst_f[:], meta_t[:, KSUB:2 * KSUB])
                x_win = wp.tile([P, IN_DIM], FP, tag="xwin")
                nc.gpsimd.indirect_dma_start(
                    out=x_win[:], out_offset=None, in_=xw_d[:, :],
                    in_offset=bass.IndirectOffsetOnAxis(
                        ap=meta_t[:, 2 * KSUB:2 * KSUB + 1], axis=0))
                # xr for this window's nodes: (x_win @ W_r)
                ptx = ptr.tile([IN_DIM, P], FP, tag="tr2")
                nc.tensor.transpose(ptx[:IN_DIM, :P], x_win[:, :], ident[:, :])
                xwT = wp.tile([IN_DIM, P], FR, tag="xwT")
                nc.vector.tensor_copy(xwT[:IN_DIM, :], ptx[:IN_DIM, :])
                pxr = pmm.tile([P, F + HEADS], FP, tag="mm")
                nc.tensor.matmul(pxr[:, :F], lhsT=xwT[:IN_DIM, :],
                                 rhs=wr_sb[:, :],
                                 start=True, stop=True)
                xr_sb = wp.tile([P, F], FR, tag="xr")
                nc.vector.tensor_copy(xr_sb[:], pxr[:, :F])
                eat_sb = wp.tile([EDGE_DIM, EPW], FR, tag="eat")
                nc.sync.dma_start(eat_sb[:EDGE_DIM, :], eat_d[w, :, :])
                # gather xl rows (HW indirect DMA consumes one index per
                # dest partition, so one gather per 128-edge subtile)
                xl_win = wp.tile([P, KSUB * F], FR, tag="xlwin")
                for j in range(KSUB):
                    nc.gpsimd.indirect_dma_start(
                        out=xl_win[:, j * F:(j + 1) * F], out_offset=None,
                        in_=xl_t_d[:, :],
                        in_offset=bass.IndirectOffsetOnAxis(
                            ap=meta_t[:, j:j + 1], axis=0))

                outp = pout.tile([P, F + HEADS], FP, tag="out")
                for jj in range(0, KSUB, 4):
                    # one-hot (edge x node-slot) for four subtiles at once
                    ohT2 = sp.tile([P, 4 * P], FR, tag="ohT")
                    nc.vector.tensor_tensor(
                        out=ohT2[:].rearrange("p (t n) -> p t n", t=4),
                        in0=dst_f[:, jj:jj + 4, None].to_broadcast([P, 4, P]),
                        in1=iota4[:].rearrange("p (t n) -> p t n", t=4),
                        op=ALU.is_equal)
                    poh2 = ptr.tile([P, 4 * P], FR, tag="tr2")
                    for t in range(4):
                        nc.tensor.transpose(poh2[:, t * P:(t + 1) * P],
                                            ohT2[:, t * P:(t + 1) * P],
                                            ident_r[:, :])
                    oh2 = sp.tile([P, 4 * P], FR, tag="oh")
                    nc.scalar.copy(oh2[:], poh2[:, :])
                    for j in range(jj, jj + 4):
                        t = j - jj
                        zp = pmm.tile([P, F + HEADS], FP, tag="mm")
                        nc.tensor.matmul(
                            zp[:, :F],
                            lhsT=eat_sb[:EDGE_DIM,
                                        j * P:(j + 1) * P],
                            rhs=we_sb[:, :],
                            start=True, stop=False)
                        nc.tensor.matmul(
                            zp[:, :F],
                            lhsT=oh2[:, t * P:(t + 1) * P],
                            rhs=xr_sb[:, :],
                            start=False, stop=False)
                        nc.tensor.matmul(
                            zp[:, :F], lhsT=ident_r[:, :],
                            rhs=xl_win[:, j * F:(j + 1) * F],
                            start=False, stop=True)
                        z2 = sp.tile([P, F], BF, tag="z2")
                        if use_prelu:
                            nc.scalar.activation(z2[:], zp[:, :F], ACT.Prelu,
                                                 bias=czero[:, :1],
                                                 alpha=NEG_SLOPE)
                        else:
                            z2a = sp.tile([P, F], FP, tag="z2a")
                            nc.vector.tensor_scalar_mul(z2a[:], zp[:, :F],
                                                        NEG_SLOPE)
                            nc.vector.tensor_tensor(out=z2[:], in0=zp[:, :F],
                                                    in1=z2a[:], op=ALU.max)
                        tsc = sp.tile([P, F], BF, tag="tsc")
                        nc.vector.tensor_tensor(out=tsc[:], in0=z2[:],
                                                in1=att_bf[:], op=ALU.mult)
                        lg = sp.tile([P, HEADS], FP, tag="lg")
                        nc.vector.tensor_reduce(
                            out=lg[:],
                            in_=tsc[:].rearrange("p (h c) -> p h c", h=HEADS),
                            axis=AX.X, op=ALU.add)
                        rhs = sp.tile([P, F + HEADS], FR, tag="rhs")
                        exf = sp.tile([P, HEADS], FP, tag="exf")
                        nc.scalar.activation(exf[:], lg[:], ACT.Exp,
                                             bias=cshift[:, :1])
                        nc.vector.tensor_copy(rhs[:, F:F + HEADS], exf[:])
                        # msg = xl * alpha-numerator, split across DVE (heads
                        # 0-1) and ACT (heads 2-3) to balance the engines
                        nc.vector.tensor_tensor(
                            out=rhs[:, 0:2 * HID].rearrange(
                                "p (h c) -> p h c", h=2),
                            in0=xl_win[:, j * F:j * F + 2 * HID].rearrange(
                                "p (h c) -> p h c", h=2),
                            in1=exf[:, 0:2, None].to_broadcast([P, 2, HID]),
                            op=ALU.mult)
                        for h in (2, 3):
                            nc.scalar.mul(
                                rhs[:, h * HID:(h + 1) * HID],
                                xl_win[:, j * F + h * HID:
                                       j * F + (h + 1) * HID],
                                exf[:, h:h + 1])
                        nc.tensor.matmul(
                            outp[:, :],
                            lhsT=ohT2[:, t * P:(t + 1) * P],
                            rhs=rhs[:, :],
                            start=(j == 0), stop=(j == KSUB - 1))

                # ---------------- window epilogue ----------------
                dn = epp.tile([P, HEADS], FP, tag="dn")
                nc.vector.tensor_scalar_add(dn[:], outp[:, F:F + HEADS],
                                            DENOM_TINY)
                rec = epp.tile([P, HEADS], FP, tag="rec")
                nc.vector.reciprocal(rec[:], dn[:])
                rec2 = epp.tile([P, HEADS], FP, tag="rec2")
                nc.vector.tensor_scalar_mul(rec2[:], rec[:], 1.0 / HEADS)
                outn = epp.tile([P, F], FP, tag="outn")
                for h in range(HEADS):
                    nc.vector.tensor_scalar(
                        out=outn[:, h * HID:(h + 1) * HID],
                        in0=outp[:, h * HID:(h + 1) * HID],
                        scalar1=rec2[:, h:h + 1], scalar2=None, op0=ALU.mult)
                t1 = epp.tile([P, IN_DIM], FP, tag="t1")
                nc.vector.tensor_tensor(out=t1[:], in0=outn[:, 0:64],
                                        in1=outn[:, 64:128], op=ALU.add)
                t2 = epp.tile([P, IN_DIM], FP, tag="t2")
                nc.vector.tensor_tensor(out=t2[:], in0=outn[:, 128:192],
                                        in1=outn[:, 192:256], op=ALU.add)
                hm = epp.tile([P, IN_DIM], FP, tag="hm")
                nc.vector.tensor_tensor(out=hm[:], in0=t1[:], in1=t2[:],
                                        op=ALU.add)
                r1 = epp.tile([P, IN_DIM], FP, tag="r1")
                nc.vector.tensor_tensor(out=r1[:], in0=hm[:], in1=x_win[:],
                                        op=ALU.add)
                r2 = epp.tile([P, IN_DIM], FP, tag="r2")
                nc.vector.tensor_tensor(out=r2[:], in0=r1[:], in1=bias_rep[:],
                                        op=ALU.add)
                mus = epp.tile([P, 1], FP, tag="mus")
                nc.vector.reduce_sum(out=mus[:], in_=r2[:], axis=AX.X)
                mu64 = epp.tile([P, 1], FP, tag="mu64")
                nc.scalar.mul(mu64[:], mus[:], 1.0 / IN_DIM)
                d = epp.tile([P, IN_DIM], FP, tag="d")
                nc.vector.tensor_scalar(out=d[:], in0=r2[:],
                                        scalar1=mu64[:, :1], scalar2=None,
                                        op0=ALU.subtract)
                dsc = epp.tile([P, IN_DIM], FP, tag="dsc")
                nc.vector.tensor_tensor(out=dsc[:], in0=d[:], in1=d[:],
                                        op=ALU.mult)
                vpe = epp.tile([P, 1], FP, tag="vpe")
                nc.vector.reduce_sum(out=vpe[:], in_=dsc[:], axis=AX.X)
                # rstd = (var+eps)^-0.5 = exp(-0.5*ln(vpe/64 + eps))
                lnv = epp.tile([P, 1], FP, tag="lnv")
                nc.scalar.activation(lnv[:], vpe[:], ACT.Ln,
                                     bias=ceps[:, :1], scale=1.0 / IN_DIM)
                rstd = epp.tile([P, 1], FP, tag="rstd")
                nc.scalar.activation(rstd[:], lnv[:], ACT.Exp,
                                     bias=czero[:, :1], scale=-0.5)
                y = epp.tile([P, IN_DIM], FP, tag="y")
                nc.vector.tensor_scalar(out=y[:], in0=d[:],
                                        scalar1=rstd[:, :1], scalar2=None,
                                        op0=ALU.mult)
                y2 = epp.tile([P, IN_DIM], FP, tag="y2")
                nc.vector.tensor_tensor(out=y2[:], in0=y[:], in1=lnw_rep[:],
                                        op=ALU.mult)
                y3 = epp.tile([P, IN_DIM], FP, tag="y3")
                nc.vector.tensor_tensor(out=y3[:], in0=y2[:], in1=lnb_rep[:],
                                        op=ALU.add)
                nc.sync.dma_start(out_d[w * P:(w + 1) * P, :], y3[:])

    nc.finalize()
    return _patch_serialization(nc)


# --------------------------------------------------------------------------
# entry point
# --------------------------------------------------------------------------

_NC_CACHE = {}


def make_in_maps(inputs, prep):
    x = np.ascontiguousarray(np.asarray(inputs["x"], np.float32))
    att2 = np.ascontiguousarray(
        np.asarray(inputs["att"], np.float32).reshape(1, F))
    xt = np.ascontiguousarray(x.T)
    iotar = np.tile(np.arange(P), 4).astype(np.float32).reshape(1, 4 * P)
    in_maps = []
    for c in range(NCORES):
        in_maps.append(dict(
            xw=x,
            xt=xt,
            wl=np.ascontiguousarray(np.asarray(inputs["W_l"], np.float32)),
            wr=np.ascontiguousarray(np.asarray(inputs["W_r"], np.float32)),
            we=np.ascontiguousarray(np.asarray(inputs["W_e"], np.float32)),
            att2=att2,
            iotar=iotar,
            iotac=np.arange(P, dtype=np.float32).reshape(P, 1),
            cbias=np.asarray(inputs["bias"], np.float32).reshape(1, IN_DIM),
            clnw=np.asarray(inputs["ln_w"], np.float32).reshape(1, IN_DIM),
            clnb=np.asarray(inputs["ln_b"], np.float32).reshape(1, IN_DIM),
            meta=np.ascontiguousarray(prep["meta"][c]),
            gidx=np.ascontiguousarray(prep["gidx"][c]),
            eat=np.ascontiguousarray(prep["eat"][c]),
        ))
    return in_maps


def assemble(prep, outs):
    full = np.zeros((N, IN_DIM), np.float32)
    W = prep["meta"].shape[1]
    for c in range(NCORES):
        o = np.asarray(outs[c]).reshape(W, P, IN_DIM)
        m = prep["win_nodes_m"][c]
        sel = m >= 0
        full[m[sel]] = o[sel]
    return full


def kernel_run(inputs, trace=False, use_prelu=True):
    prep = host_prep(inputs["edge_index"], inputs["edge_attr"])
    W = int(prep["meta"].shape[1])
    key = (W, use_prelu)
    if key not in _NC_CACHE:
        _NC_CACHE[key] = build_nc(W, use_prelu=use_prelu)
    nc = _NC_CACHE[key]
    in_maps = make_in_maps(inputs, prep)
    br = run_bass_kernel_spmd(nc, in_maps, list(range(NCORES)), trace=trace)
    outs = [br.results[c]["out"] for c in range(NCORES)]
    return assemble(prep, outs), br


def kernel(**inputs):
    out, _ = kernel_run(inputs)
    return out


# revision 56
# speedup vs baseline: 1.0281x; 1.0281x over previous
"""Trainium2 Bass kernel for the EnhancedGATBlock problem.

Strategy (node/window sharded, no collectives):
  - Host sorts edges by dst and greedily packs consecutive dst-nodes into
    "windows" of <=128 nodes and <=KSUB*128 edges.  Every incoming edge of a
    node lives in exactly one window, so each window's segment-softmax and
    aggregation are fully local.
  - Windows are dealt round-robin onto 8 NeuronCores; every core runs an
    IDENTICAL static schedule of W windows x KSUB subtiles of 128 edges
    (required because run_bass_kernel_spmd compiles one SPMD program).  All
    data-dependence (edge->window assignment, node ids, per-window slots) is
    carried in index arrays, never in the IR.
  - Softmax uses a fixed shift C instead of the per-node max; alpha is
    mathematically invariant to the shift and exp stays comfortably inside
    f32 range for this data distribution (logits ~ [-12, 12]).
  - Per subtile on device: indirect-DMA gather of xl[src] rows, one-hot
    (edge x node-slot) matrix built by iota-compare, then one PSUM
    accumulation group of three matmuls (ee from host-transposed edge_attr,
    one-hot xr-expand, identity-add of xl), leaky-relu + att-dot + exp, and
    a single scatter matmul accumulating [nodes, msg|denom] into PSUM.
"""
import numpy as np

import concourse.bass as bass
import concourse.tile as tile
import concourse.mybir as mybir
from concourse import library_config
from concourse.bass_utils import run_bass_kernel_spmd

# ---- problem constants (hardcoded per the grading contract) ----
N, E = 50000, 800000
IN_DIM, HID, HEADS, EDGE_DIM = 64, 64, 4, 32
F = HEADS * HID            # 256
NEG_SLOPE = 0.2
LN_EPS = 1e-5

P = 128
NCORES = 8
KSUB = 16                  # subtiles (of 128 edges) per window
EPW = P * KSUB             # edges per window
C_SHIFT = 12.0             # fixed softmax shift (see module docstring)
DENOM_TINY = 1e-30         # guards 0-degree / pad node slots against 0/0
NMETA = 2 * KSUB + 1       # src idx | dst slot | window node id

FP = mybir.dt.float32
FR = mybir.dt.float32r     # fast PE path (1 cycle/row at N>=256)
BF = mybir.dt.bfloat16
I32 = mybir.dt.int32
ALU = mybir.AluOpType
ACT = mybir.ActivationFunctionType
AX = mybir.AxisListType


# --------------------------------------------------------------------------
# host-side prep
# --------------------------------------------------------------------------

def _pack_windows_ab(degA, degB, half_cap):
    """Greedy pack consecutive nodes into windows such that each window has
    <=128 nodes, <=half_cap edges with src in table-half A and likewise for
    half B (each half is gathered by one int16 dma_gather call)."""
    wins = []
    cur_nodes = 0
    ca = 0
    cb = 0
    start = 0
    for n in range(len(degA)):
        da, db = int(degA[n]), int(degB[n])
        assert da <= half_cap and db <= half_cap
        if cur_nodes + 1 > P or ca + da > half_cap or cb + db > half_cap:
            wins.append((start, n))
            start = n
            cur_nodes, ca, cb = 0, 0, 0
        cur_nodes += 1
        ca += da
        cb += db
    wins.append((start, len(degA)))
    return wins


def host_prep(edge_index, edge_attr, n_nodes=N):
    half_cap = EPW // 2                     # 1024 edges per table half
    HSPLIT = n_nodes // 2                   # xl table split row (int16 range)
    src = np.asarray(edge_index[0]).astype(np.int64)
    dst = np.asarray(edge_index[1]).astype(np.int64)
    # sort edges by (dst, src-half) so each window is [A-edges | B-edges]
    is_b = (src >= HSPLIT).astype(np.int64)
    order = np.lexsort((is_b, dst))
    dst_s = dst[order]
    is_b_s = is_b[order]
    deg = np.bincount(dst_s, minlength=n_nodes)
    degB = np.bincount(dst_s[is_b_s == 1], minlength=n_nodes)
    degA = deg - degB
    node_edge_start = np.concatenate([[0], np.cumsum(deg)])
    wins = _pack_windows_ab(degA, degB, half_cap)
    WT = len(wins)
    W = (WT + NCORES - 1) // NCORES

    GW = half_cap // 16                     # idx cols per half (wrapped by 16)
    meta = np.zeros((NCORES, W, P, NMETA), np.int32)
    meta[:, :, :, KSUB:2 * KSUB] = -1       # dst slot pad -> no OH match
    gidx = np.zeros((NCORES, W, 16, 2 * GW), np.int16)
    eat = np.zeros((NCORES, W, EDGE_DIM, EPW), np.float32)
    win_nodes_m = np.full((NCORES, W, P), -1, np.int64)  # pad -> -1

    edge_attr = np.asarray(edge_attr, np.float32)

    for widx, (a, b) in enumerate(wins):
        c = widx % NCORES
        w = widx // NCORES
        es, ee_ = int(node_edge_start[a]), int(node_edge_start[b])
        pe = order[es:ee_]
        bmask = is_b_s[es:ee_] == 1
        peA, peB = pe[~bmask], pe[bmask]
        nA, nB = len(peA), len(peB)
        # window-local edge positions: A block then B block at half_cap
        k = np.concatenate([np.arange(nA), half_cap + np.arange(nB)])
        pe2 = np.concatenate([peA, peB])
        p_pos = k % P
        j_pos = k // P
        meta[c, w, p_pos, j_pos] = src[pe2].astype(np.int32)
        meta[c, w, p_pos, KSUB + j_pos] = (dst[pe2] - a).astype(np.int32)
        # wrapped int16 gather indices (idx for slot k lives at [k%16, k//16])
        ia = (src[peA]).astype(np.int16)
        ib = (src[peB] - HSPLIT).astype(np.int16)
        gidx[c, w, np.arange(nA) % 16, np.arange(nA) // 16] = ia
        gidx[c, w, np.arange(nB) % 16, GW + np.arange(nB) // 16] = ib
        # c,w,k advanced indices with a slice between -> result dims are
        # (cnt, EDGE_DIM), matching edge_attr[pe2] directly
        eat[c, w, :, k] = edge_attr[pe2]
        nn = b - a
        meta[c, w, :nn, 2 * KSUB] = np.arange(a, b, dtype=np.int32)
        win_nodes_m[c, w, :nn] = np.arange(a, b)

    # dma_gather reads its wrapped index block from 128 partitions (the
    # 16-partition pattern replicated for the 8 gpsimd cores)
    gidx = np.tile(gidx, (1, 1, 8, 1))
    return dict(meta=meta, gidx=gidx, eat=eat, win_nodes_m=win_nodes_m,
                W=W, WT=WT, hsplit=HSPLIT)


# --------------------------------------------------------------------------
# BIR sync-wait legalization
# --------------------------------------------------------------------------
# walrus codegen accepts only ONE semaphore wait per ISA instruction, but
# Tile freely attaches more.  Keep the first wait on the instruction and move
# the excess onto preceding same-engine Drains (engines execute their stream
# in order, so the semantics are unchanged).

_SPILL_OPCODE = "Drain"


def legalize_sync_waits(bir_bytes):
    import orjson
    bir = orjson.loads(bir_bytes)
    n_new = 0
    for fn in bir["functions"]:
        for blk in fn["blocks"]:
            insts = blk.get("instructions")
            if not insts:
                continue
            out = []
            changed = False
            for ins in insts:
                si = ins.get("sync_info")
                waits = (si or {}).get("on_wait") or []
                if len(waits) > 1:
                    for wt in waits[1:]:
                        spill = {
                            "name": f"I-lsw{n_new}",
                            "opcode": _SPILL_OPCODE,
                            "engine": ins["engine"],
                            "ins": [],
                            "outs": [],
                            "sync_info": {"on_update": [], "on_wait": [wt]},
                        }
                        if "debug" in ins:
                            spill["debug"] = ins["debug"]
                        n_new += 1
                        out.append(spill)
                    si["on_wait"] = waits[:1]
                    changed = True
                out.append(ins)
            if changed:
                blk["instructions"] = out
    return orjson.dumps(bir)


def _patch_serialization(nc):
    orig = nc.to_json_bytes

    def patched():
        return legalize_sync_waits(orig())

    nc.to_json_bytes = patched
    return nc


# --------------------------------------------------------------------------
# device kernel
# --------------------------------------------------------------------------

def build_nc(W, n_nodes=N, use_prelu=True):
    nc = bass.Bass()
    xw_d = nc.declare_dram_parameter("xw", [n_nodes, IN_DIM], FP, isOutput=False)
    xt_d = nc.declare_dram_parameter("xt", [IN_DIM, n_nodes], FR, isOutput=False)
    wl_d = nc.declare_dram_parameter("wl", [IN_DIM, F], FP, isOutput=False)
    wr_d = nc.declare_dram_parameter("wr", [IN_DIM, F], FP, isOutput=False)
    we_d = nc.declare_dram_parameter("we", [EDGE_DIM, F], FP, isOutput=False)
    att_d = nc.declare_dram_parameter("att2", [1, F], FP, isOutput=False)
    cb_d = nc.declare_dram_parameter("cbias", [1, IN_DIM], FP, isOutput=False)
    cw_d = nc.declare_dram_parameter("clnw", [1, IN_DIM], FP, isOutput=False)
    clb_d = nc.declare_dram_parameter("clnb", [1, IN_DIM], FP, isOutput=False)
    ior_d = nc.declare_dram_parameter("iotar", [1, 4 * P], FP, isOutput=False)
    ioc_d = nc.declare_dram_parameter("iotac", [P, 1], FP, isOutput=False)
    meta_d = nc.declare_dram_parameter("meta", [W, P, NMETA], I32, isOutput=False)
    xwin_d = nc.declare_dram_parameter("xwin", [W, P, IN_DIM], FP,
                                       isOutput=False)
    xwt_d = nc.declare_dram_parameter("xwt", [W, IN_DIM, P], FR,
                                      isOutput=False)
    GW = (EPW // 2) // 16
    gid_d = nc.declare_dram_parameter("gidx", [W, P, 2 * GW], mybir.dt.int16,
                                      isOutput=False)
    eat_d = nc.declare_dram_parameter("eat", [W, EDGE_DIM, EPW], FR,
                                      isOutput=False)
    out_d = nc.declare_dram_parameter("out", [W * P, IN_DIM], FP, isOutput=True)
    xl_t_d = nc.dram_tensor("xl_table", [n_nodes, F], FR)

    with tile.TileContext(nc) as tc:
        with (
            tc.tile_pool(name="const", bufs=1) as cp,
            tc.tile_pool(name="win", bufs=3) as wp,
            tc.tile_pool(name="xlp", bufs=3) as xlp,
            tc.tile_pool(name="sub", bufs=4) as sp,
            tc.tile_pool(name="ep", bufs=3) as epp,
            tc.tile_pool(name="ptr", bufs=3, space="PSUM") as ptr,
            tc.tile_pool(name="pmm", bufs=3, space="PSUM") as pmm,
            tc.tile_pool(name="pout", bufs=2, space="PSUM") as pout,
        ):
            # ---------------- constants ----------------
            # Launder everything a matmul consumes through DVE so PE deps
            # collapse onto one semaphore (one-wait rule, see legalizer).
            def laundered(dram_ap, pdim, ncols, name, dt=FP):
                raw = cp.tile([pdim, ncols], FP, tag=name + "_r")
                nc.sync.dma_start(raw[:pdim, :], dram_ap)
                cl = cp.tile([pdim, ncols], dt, tag=name)
                nc.vector.tensor_copy(cl[:pdim, :], raw[:pdim, :])
                return cl

            wl_sb = laundered(wl_d[:, :], IN_DIM, F, "wl", dt=FR)
            wr_sb = laundered(wr_d[:, :], IN_DIM, F, "wr", dt=FR)
            we_sb = laundered(we_d[:, :], EDGE_DIM, F, "we", dt=FR)
            ior_sb = laundered(ior_d[:, :], 1, 4 * P, "ior")
            ioc_sb = laundered(ioc_d[:, :], P, 1, "ioc")
            ones1 = cp.tile([1, P], FP)
            nc.vector.memset(ones1[:], 1.0)

            def pbcast(src1, ncols, name):
                pb = pmm.tile([P, 4 * P], FP, tag="mm")
                nc.tensor.matmul(pb[:, :ncols], lhsT=ones1[:1, :],
                                 rhs=src1[:1, :ncols], start=True, stop=True)
                dst = cp.tile([P, ncols], FP, tag=name)
                nc.vector.tensor_copy(dst[:], pb[:, :ncols])
                return dst

            iota4 = pbcast(ior_sb, 4 * P, "iota4")   # [128, 512] four iotas
            ident = cp.tile([P, P], FP)
            nc.vector.tensor_tensor(out=ident[:],
                                    in0=ioc_sb[:, :1].to_broadcast([P, P]),
                                    in1=iota4[:, :P], op=ALU.is_equal)
            ident_r = cp.tile([P, P], FR)
            nc.vector.tensor_copy(ident_r[:], ident[:])
            att_rep = pbcast(laundered(att_d[:, :], 1, F, "att1"), F, "att_rep")
            att_bf = cp.tile([P, F], BF)
            nc.vector.tensor_copy(att_bf[:], att_rep[:])
            bias_rep = pbcast(laundered(cb_d[:, :], 1, IN_DIM, "b1"), IN_DIM,
                              "bias_rep")
            lnw_rep = pbcast(laundered(cw_d[:, :], 1, IN_DIM, "w1"), IN_DIM,
                             "lnw_rep")
            lnb_rep = pbcast(laundered(clb_d[:, :], 1, IN_DIM, "lb1"), IN_DIM,
                             "lnb_rep")
            czero = cp.tile([P, 1], FP)
            nc.vector.memset(czero[:], 0.0)
            cshift = cp.tile([P, 1], FP)
            nc.vector.memset(cshift[:], -C_SHIFT)
            ceps = cp.tile([P, 1], FP)
            nc.vector.memset(ceps[:], LN_EPS)

            # ---------------- prologue: xl table = x @ W_l ----------------
            # x arrives pre-transposed from the host, so each 128-node slice
            # is a direct lhsT; 512 nodes per DMA in and out.
            BN = 4 * P
            for b0 in range(0, n_nodes, BN):
                bcnt = min(BN, n_nodes - b0)
                nk = (bcnt + P - 1) // P
                xt_sb = xlp.tile([IN_DIM, BN], FR, tag="xts")
                nc.sync.dma_start(xt_sb[:IN_DIM, :bcnt],
                                  xt_d[:, b0:b0 + bcnt])
                xlo = xlp.tile([P, 4 * F], FR, tag="xlo")
                for k in range(nk):
                    cnt = min(P, bcnt - k * P)
                    pz = pmm.tile([P, F + HEADS], FP, tag="mm")
                    nc.tensor.matmul(
                        pz[:cnt, :F],
                        lhsT=xt_sb[:IN_DIM, k * P:k * P + cnt],
                        rhs=wl_sb[:, :], start=True, stop=True)
                    nc.vector.tensor_copy(xlo[:cnt, k * F:(k + 1) * F],
                                          pz[:cnt, :F])
                if bcnt == BN:
                    nc.sync.dma_start(
                        xl_t_d[b0:b0 + BN, :].rearrange("(k p) f -> p k f",
                                                        p=P),
                        xlo[:].rearrange("p (k f) -> p k f", k=4))
                else:
                    for k in range(nk):
                        cnt = min(P, bcnt - k * P)
                        nc.sync.dma_start(
                            xl_t_d[b0 + k * P:b0 + k * P + cnt, :],
                            xlo[:cnt, k * F:(k + 1) * F])

            # No barrier here: the first xl gather RAW-depends on table
            # stores across many DMA lanes, and legalize_sync_waits spills
            # the extra waits into a Drain chain. Leaving the boundary open
            # lets Tile overlap the DMA-bound table build with early window
            # work (one-hots, xr, edge-attr loads).

            # ---------------- main loop over windows ----------------
            for w in range(W):
                meta_t = wp.tile([P, NMETA], I32, tag="meta")
                nc.sync.dma_start(meta_t[:], meta_d[w, :, :])
                dst_f = wp.tile([P, KSUB], FP, tag="dstf")
                nc.vector.tensor_copy(dst_f[:], meta_t[:, KSUB:2 * KSUB])
                # window nodes are consecutive, so the host ships x[a:b]
                # (and its transpose, as the W_r matmul's lhsT) directly --
                # no gather / on-device transpose needed.
                x_win = wp.tile([P, IN_DIM], FP, tag="xwin")
                nc.sync.dma_start(x_win[:], xwin_d[w, :, :])
                xwT = wp.tile([IN_DIM, P], FR, tag="xwT")
                nc.sync.dma_start(xwT[:IN_DIM, :], xwt_d[w, :, :])
                pxr = pmm.tile([P, F + HEADS], FP, tag="mm")
                nc.tensor.matmul(pxr[:, :F], lhsT=xwT[:IN_DIM, :],
                                 rhs=wr_sb[:, :],
                                 start=True, stop=True)
                xr_sb = wp.tile([P, F], FR, tag="xr")
                nc.vector.tensor_copy(xr_sb[:], pxr[:, :F])
                eat_sb = wp.tile([EDGE_DIM, EPW], FR, tag="eat")
                nc.sync.dma_start(eat_sb[:EDGE_DIM, :], eat_d[w, :, :])
                # gather xl rows (HW indirect DMA consumes one index per
                # dest partition, so one gather per 128-edge subtile)
                xl_win = wp.tile([P, KSUB * F], FR, tag="xlwin")
                for j in range(KSUB):
                    nc.gpsimd.indirect_dma_start(
                        out=xl_win[:, j * F:(j + 1) * F], out_offset=None,
                        in_=xl_t_d[:, :],
                        in_offset=bass.IndirectOffsetOnAxis(
                            ap=meta_t[:, j:j + 1], axis=0))

                outp = pout.tile([P, F + HEADS], FP, tag="out")
                for jj in range(0, KSUB, 4):
                    # one-hot (edge x node-slot) for four subtiles at once
                    ohT2 = sp.tile([P, 4 * P], FR, tag="ohT")
                    nc.vector.tensor_tensor(
                        out=ohT2[:].rearrange("p (t n) -> p t n", t=4),
                        in0=dst_f[:, jj:jj + 4, None].to_broadcast([P, 4, P]),
                        in1=iota4[:].rearrange("p (t n) -> p t n", t=4),
                        op=ALU.is_equal)
                    poh2 = ptr.tile([P, 4 * P], FR, tag="tr2")
                    for t in range(4):
                        nc.tensor.transpose(poh2[:, t * P:(t + 1) * P],
                                            ohT2[:, t * P:(t + 1) * P],
                                            ident_r[:, :])
                    oh2 = sp.tile([P, 4 * P], FR, tag="oh")
                    nc.scalar.copy(oh2[:], poh2[:, :])
                    for j in range(jj, jj + 4):
                        t = j - jj
                        zp = pmm.tile([P, F + HEADS], FP, tag="mm")
                        nc.tensor.matmul(
                            zp[:, :F],
                            lhsT=eat_sb[:EDGE_DIM,
                                        j * P:(j + 1) * P],
                            rhs=we_sb[:, :],
                            start=True, stop=False)
                        nc.tensor.matmul(
                            zp[:, :F],
                            lhsT=oh2[:, t * P:(t + 1) * P],
                            rhs=xr_sb[:, :],
                            start=False, stop=False)
                        nc.tensor.matmul(
                            zp[:, :F], lhsT=ident_r[:, :],
                            rhs=xl_win[:, j * F:(j + 1) * F],
                            start=False, stop=True)
                        z2 = sp.tile([P, F], BF, tag="z2")
                        if use_prelu:
                            nc.scalar.activation(z2[:], zp[:, :F], ACT.Prelu,
                                                 bias=czero[:, :1],
                                                 alpha=NEG_SLOPE)
                        else:
                            z2a = sp.tile([P, F], FP, tag="z2a")
                            nc.vector.tensor_scalar_mul(z2a[:], zp[:, :F],
                                                        NEG_SLOPE)
                            nc.vector.tensor_tensor(out=z2[:], in0=zp[:, :F],
                                                    in1=z2a[:], op=ALU.max)
                        tsc = sp.tile([P, F], BF, tag="tsc")
                        nc.vector.tensor_tensor(out=tsc[:], in0=z2[:],
                                                in1=att_bf[:], op=ALU.mult)
                        lg = sp.tile([P, HEADS], FP, tag="lg")
                        nc.vector.tensor_reduce(
                            out=lg[:],
                            in_=tsc[:].rearrange("p (h c) -> p h c", h=HEADS),
                            axis=AX.X, op=ALU.add)
                        rhs = sp.tile([P, F + HEADS], FR, tag="rhs")
                        exf = sp.tile([P, HEADS], FP, tag="exf")
                        nc.scalar.activation(exf[:], lg[:], ACT.Exp,
                                             bias=cshift[:, :1])
                        nc.vector.tensor_copy(rhs[:, F:F + HEADS], exf[:])
                        # msg = xl * alpha-numerator, split across DVE (heads
                        # 0-1) and ACT (heads 2-3) to balance the engines
                        nc.vector.tensor_tensor(
                            out=rhs[:, 0:2 * HID].rearrange(
                                "p (h c) -> p h c", h=2),
                            in0=xl_win[:, j * F:j * F + 2 * HID].rearrange(
                                "p (h c) -> p h c", h=2),
                            in1=exf[:, 0:2, None].to_broadcast([P, 2, HID]),
                            op=ALU.mult)
                        for h in (2, 3):
                            nc.scalar.mul(
                                rhs[:, h * HID:(h + 1) * HID],
                                xl_win[:, j * F + h * HID:
                                       j * F + (h + 1) * HID],
                                exf[:, h:h + 1])
                        nc.tensor.matmul(
                            outp[:, :],
                            lhsT=ohT2[:, t * P:(t + 1) * P],
                            rhs=rhs[:, :],
                            start=(j == 0), stop=(j == KSUB - 1))

                # ---------------- window epilogue ----------------
                dn = epp.tile([P, HEADS], FP, tag="dn")
                nc.vector.tensor_scalar_add(dn[:], outp[:, F:F + HEADS],
                                            DENOM_TINY)
                rec = epp.tile([P, HEADS], FP, tag="rec")
                nc.vector.reciprocal(rec[:], dn[:])
                rec2 = epp.tile([P, HEADS], FP, tag="rec2")
                nc.vector.tensor_scalar_mul(rec2[:], rec[:], 1.0 / HEADS)
                outn = epp.tile([P, F], FP, tag="outn")
                nc.vector.tensor_tensor(
                    out=outn[:].rearrange("p (h c) -> p h c", h=HEADS),
                    in0=outp[:, 0:F].rearrange("p (h c) -> p h c", h=HEADS),
                    in1=rec2[:, :, None].to_broadcast([P, HEADS, HID]),
                    op=ALU.mult)
                hm = epp.tile([P, IN_DIM], FP, tag="hm")
                nc.vector.tensor_reduce(
                    out=hm[:],
                    in_=outn[:].rearrange("p (h c) -> p c h", h=HEADS),
                    axis=AX.X, op=ALU.add)
                r1 = epp.tile([P, IN_DIM], FP, tag="r1")
                nc.vector.tensor_tensor(out=r1[:], in0=hm[:], in1=x_win[:],
                                        op=ALU.add)
                r2 = epp.tile([P, IN_DIM], FP, tag="r2")
                nc.vector.tensor_tensor(out=r2[:], in0=r1[:], in1=bias_rep[:],
                                        op=ALU.add)
                mus = epp.tile([P, 1], FP, tag="mus")
                nc.vector.reduce_sum(out=mus[:], in_=r2[:], axis=AX.X)
                mu64 = epp.tile([P, 1], FP, tag="mu64")
                nc.scalar.mul(mu64[:], mus[:], 1.0 / IN_DIM)
                d = epp.tile([P, IN_DIM], FP, tag="d")
                nc.vector.tensor_scalar(out=d[:], in0=r2[:],
                                        scalar1=mu64[:, :1], scalar2=None,
                                        op0=ALU.subtract)
                dsc = epp.tile([P, IN_DIM], FP, tag="dsc")
                nc.vector.tensor_tensor(out=dsc[:], in0=d[:], in1=d[:],
                                        op=ALU.mult)
                vpe = epp.tile([P, 1], FP, tag="vpe")
                nc.vector.reduce_sum(out=vpe[:], in_=dsc[:], axis=AX.X)
                # rstd = (var+eps)^-0.5 = exp(-0.5*ln(vpe/64 + eps))
                lnv = epp.tile([P, 1], FP, tag="lnv")
                nc.scalar.activation(lnv[:], vpe[:], ACT.Ln,
                                     bias=ceps[:, :1], scale=1.0 / IN_DIM)
                rstd = epp.tile([P, 1], FP, tag="rstd")
                nc.scalar.activation(rstd[:], lnv[:], ACT.Exp,
                                     bias=czero[:, :1], scale=-0.5)
                y = epp.tile([P, IN_DIM], FP, tag="y")
                nc.vector.tensor_scalar(out=y[:], in0=d[:],
                                        scalar1=rstd[:, :1], scalar2=None,
                                        op0=ALU.mult)
                y2 = epp.tile([P, IN_DIM], FP, tag="y2")
                nc.vector.tensor_tensor(out=y2[:], in0=y[:], in1=lnw_rep[:],
                                        op=ALU.mult)
                y3 = epp.tile([P, IN_DIM], FP, tag="y3")
                nc.vector.tensor_tensor(out=y3[:], in0=y2[:], in1=lnb_rep[:],
                                        op=ALU.add)
                nc.sync.dma_start(out_d[w * P:(w + 1) * P, :], y3[:])

    nc.finalize()
    return _patch_serialization(nc)


# --------------------------------------------------------------------------
# entry point
# --------------------------------------------------------------------------

_NC_CACHE = {}


def _xwin(x, prep, c):
    m = prep["win_nodes_m"][c]                    # [W, 128], -1 pads
    xw = x[np.clip(m, 0, None)] * (m >= 0)[:, :, None].astype(np.float32)
    return np.ascontiguousarray(xw)


def make_in_maps(inputs, prep):
    x = np.ascontiguousarray(np.asarray(inputs["x"], np.float32))
    att2 = np.ascontiguousarray(
        np.asarray(inputs["att"], np.float32).reshape(1, F))
    xt = np.ascontiguousarray(x.T)
    iotar = np.tile(np.arange(P), 4).astype(np.float32).reshape(1, 4 * P)
    in_maps = []
    for c in range(NCORES):
        in_maps.append(dict(
            xw=x,
            xt=xt,
            wl=np.ascontiguousarray(np.asarray(inputs["W_l"], np.float32)),
            wr=np.ascontiguousarray(np.asarray(inputs["W_r"], np.float32)),
            we=np.ascontiguousarray(np.asarray(inputs["W_e"], np.float32)),
            att2=att2,
            iotar=iotar,
            iotac=np.arange(P, dtype=np.float32).reshape(P, 1),
            cbias=np.asarray(inputs["bias"], np.float32).reshape(1, IN_DIM),
            clnw=np.asarray(inputs["ln_w"], np.float32).reshape(1, IN_DIM),
            clnb=np.asarray(inputs["ln_b"], np.float32).reshape(1, IN_DIM),
            meta=np.ascontiguousarray(prep["meta"][c]),
            xwin=_xwin(x, prep, c),
            xwt=np.ascontiguousarray(
                _xwin(x, prep, c).transpose(0, 2, 1)),
            gidx=np.ascontiguousarray(prep["gidx"][c]),
            eat=np.ascontiguousarray(prep["eat"][c]),
        ))
    return in_maps


def assemble(prep, outs):
    full = np.zeros((N, IN_DIM), np.float32)
    W = prep["meta"].shape[1]
    for c in range(NCORES):
        o = np.asarray(outs[c]).reshape(W, P, IN_DIM)
        m = prep["win_nodes_m"][c]
        sel = m >= 0
        full[m[sel]] = o[sel]
    return full


def kernel_run(inputs, trace=False, use_prelu=True):
    prep = host_prep(inputs["edge_index"], inputs["edge_attr"])
    W = int(prep["meta"].shape[1])
    key = (W, use_prelu)
    if key not in _NC_CACHE:
        _NC_CACHE[key] = build_nc(W, use_prelu=use_prelu)
    nc = _NC_CACHE[key]
    in_maps = make_in_maps(inputs, prep)
    br = run_bass_kernel_spmd(nc, in_maps, list(range(NCORES)), trace=trace)
    outs = [br.results[c]["out"] for c in range(NCORES)]
    return assemble(prep, outs), br


def kernel(**inputs):
    out, _ = kernel_run(inputs)
    return out
